# revision 1
# baseline (speedup 1.0000x reference)
"""Trainium2 Bass kernel for nn_ChannelAttention (B=4, C=256, nh=8, N=24^3).

Sharding: 8 cores = 4 batches x 2 head-halves. Each core runs an identical
Bass program on its own data slice (no collectives):
  core c -> batch b = c//2, heads hh*4..hh*4+4 (hh = c%2), i.e. 128 of the
  256 output channels of that batch.

Per-core pipeline (layouts chosen so no transposes of big tensors occur):
  phase 1: stream x (fp8 e4m3) in 128-token chunks; q/k projection and the
           Gram accumulations run as fp8 DoubleRow matmuls (contraction 256
           packed as [128,2,*]); two chunks share one [128,512] PSUM bank so
           each pair needs a single fp8 eviction:
             qk_psum[n,256] = x_chunk^T @ Wqk          (tokens on partitions)
           accumulated in PSUM over all N:
             gA = q^T q            [128,128]  (diag -> ||q_d||^2)
             gB = k^T [q | k]      [128,256]  (H=k^Tq = G^T, diag kk -> ||k_e||^2)
  softmax (tiny, 128x128): E^T[e,d] = exp(H * invq[d] * invk[e] * temp).
           inv-norms via exp(-0.5*ln(ss)) so the only ACT table set needed
           in the whole kernel is {ln, exp, copy} (preloaded at t=0 by a
           dummy Ln -> zero mid-kernel table loads). invq is replicated
           along the free axis with a PE transpose + K=1 outer-product
           matmul. A block-diagonal 0/1 mask kills cross-head terms; the
           softmax denominator Z_d = sum_e E^T[e,d] (matmul with ones) is
           folded into the phase-2 eviction as a per-partition 1/Z scale.
  phase 2: stream 512-token chunks (x also uploaded as bf16, DMA'd behind
           phase-1 compute since it is only read here):
             v_psum[128,n] = Wv^T @ x_chunk   (bf16; channels on partitions)
             out_psum = E_m^T.T @ v   (block-diag zeros make it per-head)
           evict with scale=1/Z -> bf16 -> DMA out (host upcasts to f32).

Measured (8-core TRN2 via axon): relative error 5.35e-3 vs the f32 reference;
cost-model (TimelineSim) kernel time 53.4 us per core.
"""

import os

import numpy as np
import ml_dtypes

BF16 = ml_dtypes.bfloat16
FP8 = ml_dtypes.float8_e4m3
P = 128
C = 256
NH = 8
N = 24 * 24 * 24  # 13824
B = 4
NCORES = 8
EPS = 1e-12
CHUNK1 = 128
CHUNK2 = 512
# first slab small so compute starts early; 512 + 8*1664 = 13824
SLABS = [512] + [1664] * 8

_PROGRAM_CACHE = {}
LAST_RESULTS = None  # test harness reads exec_time_ns from here


def _build_program():
    import concourse.mybir as mybir
    import concourse.tile as tile
    from concourse import bacc

    f32 = mybir.dt.float32
    bf = mybir.dt.bfloat16
    AF = mybir.ActivationFunctionType

    stage = int(os.environ.get("KERNEL_STAGE", "3"))  # debug bisect knob

    # Bias the act-table picker: the only funcs this kernel uses are
    # {Copy, Ln, Exp}. One real set (natural_log_exp_and_others) contains all
    # three, but the greedy picker matches the first set per func, splitting
    # them across two sets (4 mid-kernel 1.3us loads). Strip ln/exp from
    # every other set (ids are positional, so order/length must not change)
    # so the whole kernel runs off a single preloaded set.
    import concourse.mybir as _mybir

    _orig_tables = bacc.get_activation_tables

    def _patched_tables(arch):
        tabs = _orig_tables(arch)
        ln = _mybir.ActivationFunctionType.Ln
        ex = _mybir.ActivationFunctionType.Exp
        combined = {
            name for name, funcs in tabs.items() if ln in funcs and ex in funcs
        }
        if combined:
            keep = next(iter(combined))
            tabs = {
                name: (funcs if name == keep else funcs - {ln, ex})
                for name, funcs in tabs.items()
            }
        return tabs

    bacc.get_activation_tables = _patched_tables
    try:
        return _build_program_inner(nc_factory=lambda: bacc.Bacc(
            "TRN2", target_bir_lowering=False
        ), stage=stage)
    finally:
        bacc.get_activation_tables = _orig_tables


def _build_program_inner(nc_factory, stage):
    import concourse.mybir as mybir
    import concourse.tile as tile

    f32 = mybir.dt.float32
    bf = mybir.dt.bfloat16
    AF = mybir.ActivationFunctionType

    nc = nc_factory()

    f8 = mybir.dt.float8e4
    DR = mybir.MatmulPerfMode.DoubleRow
    xbf_d = nc.dram_tensor("xbf", [P, 2, N], bf, kind="ExternalInput")
    xq_d = nc.dram_tensor("xq", [P, 2, N], f8, kind="ExternalInput")
    wqk_d = nc.dram_tensor("wqk", [P, 2, 256], f8, kind="ExternalInput")
    wv_d = nc.dram_tensor("wv", [P, 2, P], bf, kind="ExternalInput")
    # consts[:, 0:128]=identity, [:, 128:256]=block-diag mask, [:, 256]=temp
    consts_d = nc.dram_tensor("consts", [P, 257], f32, kind="ExternalInput")
    out_d = nc.dram_tensor("out", [P, N], bf, kind="ExternalOutput")

    nch1 = N // CHUNK1
    nch2 = N // CHUNK2

    with tile.TileContext(nc) as tc:
        with tc.tile_pool(name="persist", bufs=1) as persist:
            xfull = persist.tile([P, 2, N], bf)
            xq = persist.tile([P, 2, N], f8)
            wqk = persist.tile([P, 2, 256], f8)
            wv = persist.tile([P, 2, P], bf)
            consts = persist.tile([P, 257], f32)
            ones1 = persist.tile([P, 1], bf)
            onesr = persist.tile([1, P], f32)
            dum0 = persist.tile([P, 1], f32)
            dum1 = persist.tile([P, 1], f32)
            emt = persist.tile([P, P], bf)  # masked exp(S)^T, lhsT of phase 2
            invz = persist.tile([P, 1], f32)

            ident = consts[:, 0:P]
            maskblk = consts[:, P : 2 * P]
            tempv = consts[:, 2 * P : 2 * P + 1]

            # constants + ACT table preload ({ln, exp, copy} set) at t=0
            nc.vector.memset(ones1, 1.0)
            nc.vector.memset(onesr, 1.0)
            nc.vector.memset(dum0, 1.0)
            nc.scalar.activation(dum1, dum0, AF.Ln)
            nc.scalar.activation(dum1, dum0, AF.Exp)

            # DMA order: first matmul needs wqk + slab0 only
            nc.sync.dma_start(wqk, wqk_d[:])
            slab_edges = [0]
            for s in SLABS:
                slab_edges.append(slab_edges[-1] + s)
            nc.sync.dma_start(
                xq[:, :, 0 : slab_edges[1]], xq_d[:, :, 0 : slab_edges[1]]
            )
            nc.sync.dma_start(wv, wv_d[:])
            nc.sync.dma_start(consts, consts_d[:])

            with tc.tile_pool(name="accp", bufs=1, space="PSUM") as accp:
                gA = accp.tile([P, P], f32)
                gB = accp.tile([P, 256], f32)

                with (
                    tc.tile_pool(name="p1s", bufs=6) as p1s,
                    tc.tile_pool(name="p1p", bufs=5, space="PSUM") as p1p,
                ):
                    i = 0
                    qk8 = None
                    npairs = nch1 // 2
                    for s in range(len(SLABS)):
                        if s > 0:
                            n0s, n1s = slab_edges[s], slab_edges[s + 1]
                            nc.sync.dma_start(
                                xq[:, :, n0s:n1s], xq_d[:, :, n0s:n1s]
                            )
                        for _ in range(SLABS[s] // CHUNK1):
                            n0 = i * CHUNK1
                            if i % 2 == 0:
                                qk_ps = p1p.tile([P, 2, 256], f32, tag="qkps")
                            nc.tensor.matmul(
                                qk_ps[:, i % 2, :],
                                xq[:, :, n0 : n0 + CHUNK1],
                                wqk,
                                start=True,
                                stop=True,
                                perf_mode=DR,
                            )
                            if i % 2 == 1:
                                # one [128,512] eviction per chunk pair; the
                                # psum halves line up with qk8's (ko) halves
                                qk8 = p1s.tile([P, 2, 256], f8, tag="qk8")
                                pair = i // 2
                                if pair % 2 == 0:
                                    nc.scalar.activation(qk8, qk_ps, AF.Copy)
                                else:
                                    nc.vector.tensor_copy(qk8, qk_ps)
                                st = pair == 0
                                sp = pair == npairs - 1
                                nc.tensor.matmul(
                                    gA,
                                    qk8[:, :, 0:P],
                                    qk8[:, :, 0:P],
                                    start=st,
                                    stop=sp,
                                    skip_group_check=True,
                                    perf_mode=DR,
                                )
                                nc.tensor.matmul(
                                    gB,
                                    qk8[:, :, P : 2 * P],
                                    qk8[:, :, 0 : 2 * P],
                                    start=st,
                                    stop=sp,
                                    skip_group_check=True,
                                    perf_mode=DR,
                                )
                            i += 1
                    # xbf (phase-2 consumer) after all xq slabs: it hides
                    # behind phase-1 compute and is consumed in slab order
                    for s in range(len(SLABS)):
                        n0s, n1s = slab_edges[s], slab_edges[s + 1]
                        nc.sync.dma_start(
                            xfull[:, :, n0s:n1s], xbf_d[:, :, n0s:n1s]
                        )

                if stage == 1:
                    with tc.tile_pool(name="dbg", bufs=1) as dbg:
                        gdump = dbg.tile([P, 256], bf)
                        nc.scalar.activation(gdump, gB, AF.Copy)
                        nc.sync.dma_start(out_d[:, 0:256], gdump)

                # ---- softmax block (tiny) ----
                if stage >= 2:
                    with (
                        tc.tile_pool(name="sms", bufs=1) as sms,
                        tc.tile_pool(name="smp", bufs=1, space="PSUM") as smp,
                    ):
                        scr = sms.tile([P, P], f32)
                        scr2 = sms.tile([P, P], f32)
                        ssq = sms.tile([P, 1], f32)
                        ssk = sms.tile([P, 1], f32)
                        nc.vector.tensor_mul(scr, gA, ident)
                        nc.vector.reduce_sum(ssq, scr, axis=mybir.AxisListType.X)
                        nc.vector.tensor_mul(scr2, gB[:, P : 2 * P], ident)
                        nc.vector.reduce_sum(ssk, scr2, axis=mybir.AxisListType.X)
                        # invq = 1/max(sqrt(ssq), EPS) == exp(-0.5*ln(max(ssq, EPS^2)))
                        nc.vector.tensor_scalar_max(ssq, ssq, EPS * EPS)
                        nc.vector.tensor_scalar_max(ssk, ssk, EPS * EPS)
                        lq = sms.tile([P, 1], f32)
                        lk = sms.tile([P, 1], f32)
                        nc.scalar.activation(lq, ssq, AF.Ln)
                        nc.scalar.activation(lk, ssk, AF.Ln)
                        invq = sms.tile([P, 1], f32)
                        invk = sms.tile([P, 1], f32)
                        nc.scalar.activation(invq, lq, AF.Exp, scale=-0.5)
                        nc.scalar.activation(invk, lk, AF.Exp, scale=-0.5)
                        invkt = sms.tile([P, 1], f32)
                        nc.vector.tensor_mul(invkt, invk, tempv)

                        # rep[e,d] = invq[d]: PE transpose then K=1 outer product
                        tp_ps = smp.tile([1, P], f32)
                        nc.tensor.transpose(tp_ps, invq, ident)
                        tp_sb = sms.tile([1, P], f32)
                        nc.scalar.activation(tp_sb, tp_ps, AF.Copy)
                        rep_ps = smp.tile([P, P], f32)
                        nc.tensor.matmul(rep_ps, onesr, tp_sb, start=True, stop=True)
                        rep_sb = sms.tile([P, P], f32)
                        nc.scalar.activation(rep_sb, rep_ps, AF.Copy)

                        t1 = sms.tile([P, P], f32)
                        nc.vector.tensor_mul(t1, gB[:, 0:P], rep_sb)
                        et = sms.tile([P, P], f32)
                        nc.scalar.activation(et, t1, AF.Exp, scale=invkt)
                        nc.vector.tensor_mul(emt, et, maskblk)
                        z_ps = smp.tile([P, 1], f32)
                        nc.tensor.matmul(z_ps, emt, ones1, start=True, stop=True)
                        nc.vector.reciprocal(invz, z_ps)

            # ---- phase 2 ----
            if stage >= 3:
                with (
                    tc.tile_pool(name="p2s", bufs=8) as p2s,
                    tc.tile_pool(name="p2p", bufs=4, space="PSUM") as p2p,
                ):
                    for j in range(nch2):
                        n0 = j * CHUNK2
                        v_ps = p2p.tile([P, CHUNK2], f32, tag="vps", bufs=4)
                        nc.tensor.matmul(
                            v_ps, wv[:, 0, :], xfull[:, 0, n0 : n0 + CHUNK2],
                            start=True, stop=False,
                        )
                        nc.tensor.matmul(
                            v_ps, wv[:, 1, :], xfull[:, 1, n0 : n0 + CHUNK2],
                            start=False, stop=True,
                        )
                        v_sb = p2s.tile([P, CHUNK2], bf, tag="vsb")
                        o_ps = p2p.tile([P, CHUNK2], f32, tag="ops", bufs=4)
                        o_sb = p2s.tile([P, CHUNK2], bf, tag="osb")
                        nc.vector.tensor_copy(v_sb, v_ps)
                        nc.tensor.matmul(o_ps, emt, v_sb, start=True, stop=True)
                        nc.scalar.activation(o_sb, o_ps, AF.Copy, scale=invz)
                        nc.sync.dma_start(out_d[:, n0 : n0 + CHUNK2], o_sb)

    nc.compile()
    return nc


def _get_program():
    if "nc" not in _PROGRAM_CACHE:
        _PROGRAM_CACHE["nc"] = _build_program()
    return _PROGRAM_CACHE["nc"]


def kernel(x, W_qkvv, temperature):
    global LAST_RESULTS
    from concourse.bass_utils import run_bass_kernel_spmd

    x = np.asarray(x, dtype=np.float32)
    W = np.asarray(W_qkvv, dtype=np.float32)
    temp = np.asarray(temperature, dtype=np.float32).reshape(NH)

    ident = np.eye(P, dtype=np.float32)
    maskblk = np.kron(np.eye(4, dtype=np.float32), np.ones((32, 32), np.float32))

    in_maps = []
    for core in range(NCORES):
        b = core // 2
        hh = core % 2
        xb = x[b].reshape(C, N)
        xsh = np.ascontiguousarray(
            xb.reshape(2, P, N).transpose(1, 0, 2)
        ).astype(BF16)
        qcols = W[:, 128 * hh : 128 * hh + 128]
        kcols = W[:, C + 128 * hh : C + 128 * hh + 128]
        vcols = W[:, 2 * C + 128 * hh : 2 * C + 128 * hh + 128]
        wqk = np.ascontiguousarray(
            np.concatenate([qcols, kcols], axis=1).reshape(2, P, 256).transpose(1, 0, 2)
        ).astype(FP8)
        wv = np.ascontiguousarray(
            vcols.reshape(2, P, P).transpose(1, 0, 2)
        ).astype(BF16)
        tempv = np.repeat(temp[4 * hh : 4 * hh + 4], 32).reshape(P, 1).astype(np.float32)
        consts = np.concatenate([ident, maskblk, tempv], axis=1).astype(np.float32)
        xq8 = np.ascontiguousarray(
            xb.reshape(2, P, N).transpose(1, 0, 2)
        ).astype(FP8)
        in_maps.append(
            {"xbf": xsh, "xq": xq8, "wqk": wqk, "wv": wv, "consts": consts}
        )

    nc = _get_program()
    trace = bool(int(os.environ.get("KERNEL_TRACE", "0")))
    res = run_bass_kernel_spmd(
        nc, in_maps, core_ids=list(range(NCORES)), trace=trace
    )
    LAST_RESULTS = res

    out_full = np.empty((B, C, N), np.float32)
    for core in range(NCORES):
        b = core // 2
        hh = core % 2
        out_full[b, 128 * hh : 128 * hh + 128, :] = res.results[core]["out"].astype(np.float32)
    return out_full.reshape(B, C, 24, 24, 24)

